# revision 5
# baseline (speedup 1.0000x reference)
"""Trainium2 Bass kernel for nn_EvenOddFunctionHAM.

Computes, for W = W_tensor * W_mask (block-staircase 4096x4096):
    s_odd = rho(s) @ W + b_odd;  s_odd[:, :2048] += Ux
    out   = rho(s_odd) @ W.T + b_even
with rho(x) = sigmoid(4x - 2).

Strategy: data-parallel over the batch (4096 rows -> 8 cores x 512).
Everything runs in a transposed layout (feature dim on SBUF partitions,
batch on the free axis) so no on-device transposes are needed:
    S1 = W.T @ rho(s).T   (contraction over the even dim)
    O  = W  @ rho(S1+..)  (contraction over the odd dim)
Weights are masked, cast to fp16, transposed, and pre-tiled into
contiguous per-m-tile K-strips on the host; matmuls run in fp16 with
fp32 PSUM accumulation. The staircase zero block is skipped when the
masked W actually has it (checked on host), saving 25% of the FLOPs.
"""

import numpy as np

_KERNEL_CACHE = {}

# ---- model dims (hardcoded per contract; asserted against inputs) ----
B = 4096        # batch
E = 4096        # even dim (rows of W)
O_DIM = 4096    # odd dim (cols of W)
D1 = 2048       # width of Ux / first odd block
NC = 8          # cores
BC = B // NC    # batch per core = 512
P = 128         # partitions
NKE = E // P    # 32 k-tiles over even
NKO = O_DIM // P
NM1 = O_DIM // P  # mm1 output tiles (odd)
NM2 = E // P      # mm2 output tiles (even)
HALF = D1 // P    # 16


def _split_excess_waits(nc, maxw: int = 1) -> int:
    """This walrus build encodes at most one sem wait per instruction, but
    Tile's scheduler can attach several. Move the overflow onto inserted
    same-engine NoOps directly preceding the instruction (engines are
    in-order, so consecutive waits are equivalent to one multi-wait)."""
    from concourse import mybir

    n = 0
    for f in nc.m.functions:
        for bb in f.blocks:
            insts = bb.instructions
            new = []
            for inst in insts:
                si = getattr(inst, "sync_info", None)
                if si is not None and len(si.on_wait) > maxw:
                    waits = list(si.on_wait)
                    over, keep = waits[:-maxw], waits[-maxw:]
                    for j in range(0, len(over), maxw):
                        n += 1
                        new.append(mybir.InstNoOp(
                            name=f"{inst.name}-ws{j}",
                            engine=inst.engine,
                            bass_nofuse=True,
                            sync_info=mybir.SyncInfo(
                                on_wait=over[j : j + maxw], on_update=[]
                            ),
                        ))
                    inst.sync_info = mybir.SyncInfo(
                        on_wait=keep, on_update=list(si.on_update)
                    )
                new.append(inst)
            if len(new) != len(insts):
                insts[:] = new
                assert len(bb.instructions) == len(new)
    return n


def _build(sparse: bool):
    """Build the per-core Bass program (same program on all 8 cores)."""
    import concourse.bass as bass
    import concourse.tile as tile
    from concourse import mybir

    f32 = mybir.dt.float32
    f16 = mybir.dt.float16

    nk1a = HALF if sparse else NKE   # mm1 K-tiles for odd0 m-tiles
    nk2b = HALF if sparse else NKO   # mm2 K-tiles for even1 m-tiles

    nc = bass.Bass("TRN2", target_bir_lowering=False, debug=False)

    sT = nc.dram_tensor("sT", [NKE, P, BC], f32, kind="ExternalInput")
    uT = nc.dram_tensor("uT", [HALF, P, BC], f32, kind="ExternalInput")
    w1a = nc.dram_tensor("w1a", [HALF, P, nk1a, P], f16, kind="ExternalInput")
    w1b = nc.dram_tensor("w1b", [HALF, P, NKE, P], f16, kind="ExternalInput")
    w2a = nc.dram_tensor("w2a", [HALF, P, NKO, P], f16, kind="ExternalInput")
    w2b = nc.dram_tensor("w2b", [HALF, P, nk2b, P], f16, kind="ExternalInput")
    bias1 = nc.dram_tensor("bias1", [P, NM1], f32, kind="ExternalInput")
    bias2 = nc.dram_tensor("bias2", [P, NM2], f32, kind="ExternalInput")
    out = nc.dram_tensor("o", [NM2, P, BC], f32, kind="ExternalOutput")

    with tile.TileContext(nc) as tc:
        with (
            tc.tile_pool(name="consts", bufs=1) as consts,
            tc.tile_pool(name="stage", bufs=4) as stage,
            tc.tile_pool(name="at", bufs=NKE) as at_pool,
            tc.tile_pool(name="ut", bufs=HALF) as ut_pool,
            tc.tile_pool(name="a2", bufs=NKO) as a2_pool,
            tc.tile_pool(name="wpool", bufs=3) as wpool,
            tc.tile_pool(name="psum", bufs=4, space="PSUM") as psum_pool,
            tc.tile_pool(name="opool", bufs=4) as opool,
        ):
            b1 = consts.tile([P, NM1], f32, tag="b1")
            nc.sync.dma_start(out=b1, in_=bias1[:, :])
            b2 = consts.tile([P, NM2], f32, tag="b2")
            nc.sync.dma_start(out=b2, in_=bias2[:, :])
            bneg2 = consts.tile([P, 1], f32, tag="bneg2")
            nc.vector.memset(bneg2, -2.0)

            # rho(s).T tiles, fp16, resident: AT[k] = sigmoid(4*sT[k] - 2)
            AT = []
            for k in range(NKE):
                st = stage.tile([P, BC], f32, tag="stage")
                nc.sync.dma_start(out=st, in_=sT[k])
                a = at_pool.tile([P, BC], f16, tag="at")
                nc.scalar.activation(
                    a, st, mybir.ActivationFunctionType.Sigmoid,
                    bias=bneg2[:, 0:1], scale=4.0,
                )
                AT.append(a)

            # Ux.T tiles, fp32, resident
            UT = []
            for k in range(HALF):
                u = ut_pool.tile([P, BC], f32, tag="ut")
                nc.sync.dma_start(out=u, in_=uT[k])
                UT.append(u)

            # ---- mm1: S1[odd,:] = W.T @ AT ; A2 = rho(S1 + b_odd [+ U]) ----
            # odd1 tiles first so mm2's even1 tiles (which need only A2[16:])
            # have their deps ready the moment mm1's PE stream ends.
            A2 = [None] * NM1
            for m in list(range(HALF, NM1)) + list(range(HALF)):
                if m >= HALF:
                    wt = wpool.tile([P, NKE, P], f16, tag="w")
                    nc.sync.dma_start(out=wt, in_=w1b[m - HALF])
                    ks = range(NKE)
                else:
                    wt = wpool.tile([P, nk1a, P], f16, tag="w")
                    nc.sync.dma_start(out=wt, in_=w1a[m])
                    ks = range(nk1a)
                ps = psum_pool.tile([P, BC], f32, tag="ps")
                nkl = len(ks)
                for i, k in enumerate(ks):
                    nc.tensor.matmul(
                        ps, lhsT=wt[:, i, :], rhs=AT[k],
                        start=(i == 0), stop=(i == nkl - 1),
                    )
                if m < HALF:
                    nc.vector.tensor_add(ps, ps, UT[m])
                a2 = a2_pool.tile([P, BC], f16, tag="a2")
                nc.scalar.activation(
                    a2, ps, mybir.ActivationFunctionType.Sigmoid,
                    bias=b1[:, m : m + 1], scale=4.0,
                )
                A2[m] = a2

            # ---- mm2: O[even,:] = W @ A2 + b_even ----
            for m in list(range(HALF, NM2)) + list(range(HALF)):
                if m >= HALF:
                    wt = wpool.tile([P, nk2b, P], f16, tag="w")
                    nc.sync.dma_start(out=wt, in_=w2b[m - HALF])
                    ks = range(NKO - nk2b, NKO)
                else:
                    wt = wpool.tile([P, NKO, P], f16, tag="w")
                    nc.sync.dma_start(out=wt, in_=w2a[m])
                    ks = range(NKO)
                ps = psum_pool.tile([P, BC], f32, tag="ps")
                nkl = len(ks)
                for i, k in enumerate(ks):
                    nc.tensor.matmul(
                        ps, lhsT=wt[:, i, :], rhs=A2[k],
                        start=(i == 0), stop=(i == nkl - 1),
                    )
                ot = opool.tile([P, BC], f32, tag="ot")
                nc.scalar.activation(
                    ot, ps, mybir.ActivationFunctionType.Identity,
                    bias=b2[:, m : m + 1], scale=1.0,
                )
                nc.sync.dma_start(out=out[m], in_=ot)

    _split_excess_waits(nc, 1)
    return nc


def _strips(Wsub: np.ndarray, nm: int) -> np.ndarray:
    """[K, nm*128] -> [nm, 128, K//128, 128] contiguous per-m-tile K-strips.

    strip[j, p, kt, c] = Wsub[kt*128 + p, j*128 + c], so strip[j][:, kt, :]
    is the [K=128, M=128] lhsT tile for output tile j, contraction tile kt.
    """
    K = Wsub.shape[0]
    return np.ascontiguousarray(
        Wsub.reshape(K // P, P, nm, P).transpose(2, 1, 0, 3)
    )


def kernel(Ux, s, W_tensor, b_even, b_odd, W_mask):
    from concourse.bass_utils import run_bass_kernel_spmd

    f32 = np.float32
    Ux = np.asarray(Ux, f32)
    s = np.asarray(s, f32)
    assert s.shape == (B, E) and Ux.shape == (B, D1), (s.shape, Ux.shape)

    W = np.asarray(W_tensor, f32) * np.asarray(W_mask, f32)
    sparse = not W[D1:, :D1].any()

    W16 = W.astype(np.float16)
    WT16 = np.ascontiguousarray(W16.T)

    if sparse:
        w1a = _strips(W16[:D1, :D1], HALF)
        w2b = _strips(WT16[D1:, D1:], HALF)
    else:
        w1a = _strips(W16[:, :D1], HALF)
        w2b = _strips(WT16[:, D1:], HALF)
    w1b = _strips(W16[:, D1:], HALF)
    w2a = _strips(WT16[:, :D1], HALF)

    bias1 = np.ascontiguousarray(
        (4.0 * np.asarray(b_odd, f32).reshape(-1) - 2.0).reshape(NM1, P).T
    )
    bias2 = np.ascontiguousarray(
        np.asarray(b_even, f32).reshape(-1).reshape(NM2, P).T
    )

    sT_full = np.ascontiguousarray(s.T)   # [E, B]
    uT_full = np.ascontiguousarray(Ux.T)  # [D1, B]

    in_maps = []
    for c in range(NC):
        sl = slice(c * BC, (c + 1) * BC)
        in_maps.append({
            "sT": np.ascontiguousarray(sT_full[:, sl]).reshape(NKE, P, BC),
            "uT": np.ascontiguousarray(uT_full[:, sl]).reshape(HALF, P, BC),
            "w1a": w1a, "w1b": w1b, "w2a": w2a, "w2b": w2b,
            "bias1": bias1, "bias2": bias2,
        })

    nc = _KERNEL_CACHE.get(sparse)
    if nc is None:
        nc = _build(sparse)
        _KERNEL_CACHE[sparse] = nc

    res = run_bass_kernel_spmd(nc, in_maps, core_ids=list(range(NC)))
    out_T = np.concatenate(
        [res.results[c]["o"].reshape(E, BC) for c in range(NC)], axis=1
    )  # [E, B]
    return np.ascontiguousarray(out_T.T)


# revision 6
# speedup vs baseline: 115.5990x; 115.5990x over previous
"""Trainium2 Bass kernel for nn_EvenOddFunctionHAM.

Computes, for W = W_tensor * W_mask (block-staircase 4096x4096):
    s_odd = rho(s) @ W + b_odd;  s_odd[:, :2048] += Ux
    out   = rho(s_odd) @ W.T + b_even
with rho(x) = sigmoid(4x - 2).

Strategy: data-parallel over the batch (4096 rows -> 8 cores x 512).
Everything runs in a transposed layout (feature dim on SBUF partitions,
batch on the free axis) so no on-device transposes are needed:
    S1 = W.T @ rho(s).T   (contraction over the even dim)
    O  = W  @ rho(S1+..)  (contraction over the odd dim)
Weights are masked, cast to fp16, transposed, and pre-tiled into
contiguous per-m-tile K-strips on the host; matmuls run in fp16 with
fp32 PSUM accumulation. The staircase zero block is skipped when the
masked W actually has it (checked on host), saving 25% of the FLOPs.
"""

import numpy as np

_KERNEL_CACHE = {}

# ---- model dims (hardcoded per contract; asserted against inputs) ----
B = 4096        # batch
E = 4096        # even dim (rows of W)
O_DIM = 4096    # odd dim (cols of W)
D1 = 2048       # width of Ux / first odd block
NC = 8          # cores
BC = B // NC    # batch per core = 512
P = 128         # partitions
NKE = E // P    # 32 k-tiles over even
NKO = O_DIM // P
NM1 = O_DIM // P  # mm1 output tiles (odd)
NM2 = E // P      # mm2 output tiles (even)
HALF = D1 // P    # 16


def _split_excess_waits(nc, maxw: int = 1) -> int:
    """This walrus build encodes at most one sem wait per instruction, but
    Tile's scheduler can attach several. Move the overflow onto inserted
    same-engine NoOps directly preceding the instruction (engines are
    in-order, so consecutive waits are equivalent to one multi-wait)."""
    from concourse import mybir

    n = 0
    for f in nc.m.functions:
        for bb in f.blocks:
            insts = bb.instructions
            new = []
            for inst in insts:
                si = getattr(inst, "sync_info", None)
                if si is not None and len(si.on_wait) > maxw:
                    waits = list(si.on_wait)
                    over, keep = waits[:-maxw], waits[-maxw:]
                    for j in range(0, len(over), maxw):
                        n += 1
                        new.append(mybir.InstNoOp(
                            name=f"{inst.name}-ws{j}",
                            engine=inst.engine,
                            bass_nofuse=True,
                            sync_info=mybir.SyncInfo(
                                on_wait=over[j : j + maxw], on_update=[]
                            ),
                        ))
                    inst.sync_info = mybir.SyncInfo(
                        on_wait=keep, on_update=list(si.on_update)
                    )
                new.append(inst)
            if len(new) != len(insts):
                insts[:] = new
                assert len(bb.instructions) == len(new)
    return n


def _build(sparse: bool):
    """Build the per-core Bass program (same program on all 8 cores)."""
    import concourse.bass as bass
    import concourse.tile as tile
    from concourse import mybir

    f32 = mybir.dt.float32
    f16 = mybir.dt.float16

    nk1a = HALF if sparse else NKE   # mm1 K-tiles for odd0 m-tiles
    nk2b = HALF if sparse else NKO   # mm2 K-tiles for even1 m-tiles

    nc = bass.Bass("TRN2", target_bir_lowering=False, debug=False)

    sT = nc.dram_tensor("sT", [NKE, P, BC], f32, kind="ExternalInput")
    uT = nc.dram_tensor("uT", [HALF, P, BC], f32, kind="ExternalInput")
    w1a = nc.dram_tensor("w1a", [HALF, P, nk1a, P], f16, kind="ExternalInput")
    w1b = nc.dram_tensor("w1b", [HALF, P, NKE, P], f16, kind="ExternalInput")
    w2a = nc.dram_tensor("w2a", [HALF, P, NKO, P], f16, kind="ExternalInput")
    w2b = nc.dram_tensor("w2b", [HALF, P, nk2b, P], f16, kind="ExternalInput")
    bias1 = nc.dram_tensor("bias1", [P, NM1], f32, kind="ExternalInput")
    bias2 = nc.dram_tensor("bias2", [P, NM2], f32, kind="ExternalInput")
    out = nc.dram_tensor("o", [NM2, P, BC], f32, kind="ExternalOutput")

    with tile.TileContext(nc) as tc:
        with (
            tc.tile_pool(name="consts", bufs=1) as consts,
            tc.tile_pool(name="stage", bufs=4) as stage,
            tc.tile_pool(name="at", bufs=NKE) as at_pool,
            tc.tile_pool(name="ut", bufs=HALF) as ut_pool,
            tc.tile_pool(name="a2", bufs=NKO) as a2_pool,
            tc.tile_pool(name="wpool", bufs=3) as wpool,
            tc.tile_pool(name="psum", bufs=4, space="PSUM") as psum_pool,
            tc.tile_pool(name="opool", bufs=4) as opool,
        ):
            b1 = consts.tile([P, NM1], f32, tag="b1")
            nc.sync.dma_start(out=b1, in_=bias1[:, :])
            b2 = consts.tile([P, NM2], f32, tag="b2")
            nc.sync.dma_start(out=b2, in_=bias2[:, :])
            bneg2 = consts.tile([P, 1], f32, tag="bneg2")
            nc.vector.memset(bneg2, -2.0)

            # rho(s).T tiles, fp16, resident: AT[k] = sigmoid(4*sT[k] - 2)
            AT = []
            for k in range(NKE):
                st = stage.tile([P, BC], f32, tag="stage")
                nc.sync.dma_start(out=st, in_=sT[k])
                a = at_pool.tile([P, BC], f16, tag="at")
                nc.scalar.activation(
                    a, st, mybir.ActivationFunctionType.Sigmoid,
                    bias=bneg2[:, 0:1], scale=4.0,
                )
                AT.append(a)

            # Ux.T tiles, fp32, resident
            UT = []
            for k in range(HALF):
                u = ut_pool.tile([P, BC], f32, tag="ut")
                nc.sync.dma_start(out=u, in_=uT[k])
                UT.append(u)

            # ---- mm1: S1[odd,:] = W.T @ AT ; A2 = rho(S1 + b_odd [+ U]) ----
            # odd1 tiles first so mm2's even1 tiles (which need only A2[16:])
            # have their deps ready the moment mm1's PE stream ends.
            A2 = [None] * NM1
            for m in list(range(HALF, NM1)) + list(range(HALF)):
                if m >= HALF:
                    wt = wpool.tile([P, NKE, P], f16, tag="w")
                    nc.sync.dma_start(out=wt, in_=w1b[m - HALF])
                    ks = range(NKE)
                else:
                    wt = wpool.tile([P, nk1a, P], f16, tag="w")
                    nc.sync.dma_start(out=wt, in_=w1a[m])
                    ks = range(nk1a)
                ps = psum_pool.tile([P, BC], f32, tag="ps")
                nkl = len(ks)
                for i, k in enumerate(ks):
                    nc.tensor.matmul(
                        ps, lhsT=wt[:, i, :], rhs=AT[k],
                        start=(i == 0), stop=(i == nkl - 1),
                    )
                if m < HALF:
                    nc.vector.tensor_add(ps, ps, UT[m])
                a2 = a2_pool.tile([P, BC], f16, tag="a2")
                nc.scalar.activation(
                    a2, ps, mybir.ActivationFunctionType.Sigmoid,
                    bias=b1[:, m : m + 1], scale=4.0,
                )
                A2[m] = a2

            # ---- mm2: O[even,:] = W @ A2 + b_even ----
            for m in list(range(HALF, NM2)) + list(range(HALF)):
                if m >= HALF:
                    wt = wpool.tile([P, nk2b, P], f16, tag="w")
                    nc.sync.dma_start(out=wt, in_=w2b[m - HALF])
                    ks = range(NKO - nk2b, NKO)
                else:
                    wt = wpool.tile([P, NKO, P], f16, tag="w")
                    nc.sync.dma_start(out=wt, in_=w2a[m])
                    ks = range(NKO)
                ps = psum_pool.tile([P, BC], f32, tag="ps")
                nkl = len(ks)
                for i, k in enumerate(ks):
                    nc.tensor.matmul(
                        ps, lhsT=wt[:, i, :], rhs=A2[k],
                        start=(i == 0), stop=(i == nkl - 1),
                    )
                ot = opool.tile([P, BC], f32, tag="ot")
                nc.scalar.activation(
                    ot, ps, mybir.ActivationFunctionType.Identity,
                    bias=b2[:, m : m + 1], scale=1.0,
                )
                nc.sync.dma_start(out=out[m], in_=ot)

    _split_excess_waits(nc, 1)
    return nc


def _strips(Wsub: np.ndarray, nm: int) -> np.ndarray:
    """[K, nm*128] -> [nm, 128, K//128, 128] contiguous per-m-tile K-strips.

    strip[j, p, kt, c] = Wsub[kt*128 + p, j*128 + c], so strip[j][:, kt, :]
    is the [K=128, M=128] lhsT tile for output tile j, contraction tile kt.
    """
    K = Wsub.shape[0]
    return np.ascontiguousarray(
        Wsub.reshape(K // P, P, nm, P).transpose(2, 1, 0, 3)
    )


def prepare_in_maps(inputs: dict, W: np.ndarray, sparse: bool) -> list:
    """Host-side prep: mask+cast+tile weights, transpose activations, shard."""
    f32 = np.float32
    s = np.asarray(inputs["s"], f32)
    Ux = np.asarray(inputs["Ux"], f32)
    assert s.shape == (B, E) and Ux.shape == (B, D1), (s.shape, Ux.shape)

    W16 = W.astype(np.float16)
    WT16 = np.ascontiguousarray(W16.T)

    if sparse:
        w1a = _strips(W16[:D1, :D1], HALF)
        w2b = _strips(WT16[D1:, D1:], HALF)
    else:
        w1a = _strips(W16[:, :D1], HALF)
        w2b = _strips(WT16[:, D1:], HALF)
    w1b = _strips(W16[:, D1:], HALF)
    w2a = _strips(WT16[:, :D1], HALF)

    bias1 = np.ascontiguousarray(
        (4.0 * np.asarray(inputs["b_odd"], f32).reshape(-1) - 2.0).reshape(NM1, P).T
    )
    bias2 = np.ascontiguousarray(
        np.asarray(inputs["b_even"], f32).reshape(-1).reshape(NM2, P).T
    )

    sT_full = np.ascontiguousarray(s.T)   # [E, B]
    uT_full = np.ascontiguousarray(Ux.T)  # [D1, B]

    in_maps = []
    for c in range(NC):
        sl = slice(c * BC, (c + 1) * BC)
        in_maps.append({
            "sT": np.ascontiguousarray(sT_full[:, sl]).reshape(NKE, P, BC),
            "uT": np.ascontiguousarray(uT_full[:, sl]).reshape(HALF, P, BC),
            "w1a": w1a, "w1b": w1b, "w2a": w2a, "w2b": w2b,
            "bias1": bias1, "bias2": bias2,
        })
    return in_maps


def kernel(Ux, s, W_tensor, b_even, b_odd, W_mask):
    from concourse.bass_utils import run_bass_kernel_spmd

    f32 = np.float32
    W = np.asarray(W_tensor, f32) * np.asarray(W_mask, f32)
    sparse = not W[D1:, :D1].any()

    in_maps = prepare_in_maps(
        {"s": s, "Ux": Ux, "b_odd": b_odd, "b_even": b_even}, W, sparse
    )

    nc = _KERNEL_CACHE.get(sparse)
    if nc is None:
        nc = _build(sparse)
        _KERNEL_CACHE[sparse] = nc

    res = run_bass_kernel_spmd(nc, in_maps, core_ids=list(range(NC)))
    out_T = np.concatenate(
        [res.results[c]["o"].reshape(E, BC) for c in range(NC)], axis=1
    )  # [E, B]
    return np.ascontiguousarray(out_T.T)
